# revision 7
# baseline (speedup 1.0000x reference)
"""Trainium2 Bass kernel for LocalDenseSynthesizerAttention (band C=63, H=4 heads).

Sharding: 8192 tokens (B=2 x T=4096 flattened) split contiguously across 8
cores (1024 tokens each).  Each core runs an identical program on its own
slice; batch-edge band masking and value halo padding are handled host-side
via per-core input data, so the program is uniform SPMD.

Band construction (v3, scatter-free): the normalized softmax rows pn[i, h, k]
are DMA'd into a host-zeroed DRAM pad buffer at banded offsets
(pad[i, 128 + 1024*half + 256*h + k]), then a single DmaTransposeAnt per
tile-pair reads the pad with a SKEWED access pattern (partition step =
row_pitch - 1, i.e. element (i, f) = pad[i, 128 + f - i]) which realizes the
band skew S^T[j, i] = pn[i, j - i] and the transpose in one DMA.  Off-band
positions read host-provided zeros.  DRAM strides are unrestricted (the
partition-step legality check only applies to SBUF APs), and the tile
framework tracks the DRAM write->read hazard with semaphores.

Softmax runs at tile-PAIR granularity (256 tokens) to halve fixed per-op
engine overheads; the normalize multiply runs on the otherwise-idle GpSimd
engine.  Input DMAs ride the SP HWDGE ring in need-order (w1/w2/mask/qt0,
qt1, w3/v, v/wout); pn staging + transposes are interleaved behind them on
the same ring; per-mega output stores use the Act ring.
"""

import numpy as np
import ml_dtypes

import concourse.bass as bass
import concourse.bacc as bacc
import concourse.mybir as mybir
import concourse.tile as tile
from concourse.ap import AP
from concourse import bass_utils

BF16 = mybir.dt.bfloat16
FP32 = mybir.dt.float32
NP_BF16 = ml_dtypes.bfloat16

B, T, NF = 2, 4096, 256
H, C, DK = 4, 63, 64
HALF = (C - 1) // 2  # 31
N_CORES = 8
TPC = (B * T) // N_CORES  # 1024 tokens per core
N_TILES = TPC // 128  # 8
VPAD = 1152  # parked value rows: tokens [-31, 1121) relative to core start
SW = 256  # per-head section width in the band buffer
SBW = H * SW  # 1024
NCH = SBW // 128  # 8 chunks of S^T per tile
NPAIR = N_TILES // 2  # 4
PW = 128 + 2 * SBW  # 2176 pad width per tile-pair (128-col zero head)
PPW = NPAIR * PW  # 8704 total pnpad width
AW = 256 + 252 + 63 + 512  # blobA cols: w1t | w2t | mask | qt[0:512] = 1083


def build_program(reps: int = 1):
    import contextlib

    nc = bacc.Bacc(
        "TRN2",
        target_bir_lowering=False,
        debug=False,
        enable_asserts=False,
        num_devices=N_CORES,
    )

    blobA_d = nc.dram_tensor("blobA", [128, 2, AW], BF16, kind="ExternalInput").ap()
    blobC_d = nc.dram_tensor("blobC", [128, 2, 512], BF16, kind="ExternalInput").ap()
    blobD_d = nc.dram_tensor("blobD", [128, 2, 1664], BF16, kind="ExternalInput").ap()
    pnpad_d = nc.dram_tensor("pnpad", [128, PPW], BF16, kind="ExternalInput").ap()
    outT_d = nc.dram_tensor("outT", [NF, TPC], BF16, kind="ExternalOutput").ap()

    with tile.TileContext(nc) as tc:
        with (
            tc.tile_pool(name="inp", bufs=1) as inp,
            tc.tile_pool(name="work", bufs=6) as work,
            tc.tile_pool(name="big_ps", bufs=2, space="PSUM") as big_ps,
            tc.tile_pool(name="ob_ps", bufs=1, space="PSUM") as ob_ps,
            tc.tile_pool(name="sc_ps", bufs=3, space="PSUM") as sc_ps,
            tc.tile_pool(name="x_ps", bufs=2, space="PSUM") as x_ps,
        ):
            # ---- persistent SBUF tensors --------------------------------
            sA = inp.tile([128, 2, AW], BF16, tag="sA")
            sC = inp.tile([128, 2, 512], BF16, tag="sC")  # qt[512:1024]
            sD = inp.tile([128, 2, 1664], BF16, tag="sD")  # w3t | vt | wot
            w1t = sA[:, :, 0:256]
            w2t = sA[:, :, 256:508]
            w3t = sD[:, :, 0:256]
            vt = sD[:, :, 256:1408]
            wot = sD[:, :, 1408:1664]
            qtr = inp.tile([128, 2, TPC], BF16, tag="qtr")
            vpark = inp.tile([128, 9, NF], BF16, tag="vpark")
            xt = inp.tile([128, 2, TPC], BF16, tag="xt")
            outsb = inp.tile([128, 2, TPC], BF16, tag="outsb")
            sta2 = [
                inp.tile([128, 2, NCH, 128], BF16, tag=f"sta2_{i}", name=f"sta2_{i}")
                for i in range(NPAIR)
            ]

            loop_ctx = (
                tc.For_i(0, reps, 1, hint_engines=(mybir.EngineType.PE,))
                if reps > 1
                else contextlib.nullcontext()
            )
            with loop_ctx:
                # ---- input DMAs, all on the SP ring in need-order -------
                nc.sync.dma_start(sA[:], blobA_d)
                nc.sync.dma_start(sC[:], blobC_d)
                nc.sync.dma_start(sD[:, :, 0:832], blobD_d[:, :, 0:832])
                nc.sync.dma_start(sD[:, :, 832:1664], blobD_d[:, :, 832:1664])

                def stage1_mega(m):
                    # qtr[:, mc, m*512:(m+1)*512] = relu(w1 @ q) for 512 tokens
                    qsrc = sA[:, :, 571:1083] if m == 0 else sC
                    for mc in range(2):
                        ps = big_ps.tile([128, 512], FP32, tag="big")
                        for kc in range(2):
                            nc.tensor.matmul(
                                ps[:],
                                w1t[:, kc, mc * 128 : (mc + 1) * 128],
                                qsrc[:, kc, :],
                                start=(kc == 0),
                                stop=(kc == 1),
                            )
                        nc.scalar.activation(
                            qtr[:, mc, m * 512 : (m + 1) * 512],
                            ps[:],
                            mybir.ActivationFunctionType.Relu,
                        )

                def vpark_chunk(vp):
                    # V = value @ w3.T parked at -31 offset, sections 2vp, 2vp+1
                    nv = 2 if vp < 4 else 1
                    ps = big_ps.tile([128, 512], FP32, tag="big")
                    for j in range(nv):
                        vtile = 2 * vp + j
                        for kc in range(2):
                            nc.tensor.matmul(
                                ps[:, j * 256 : (j + 1) * 256],
                                vt[:, kc, vtile * 128 : (vtile + 1) * 128],
                                w3t[:, kc, :],
                                start=(kc == 0),
                                stop=(kc == 1),
                            )
                    dst = vpark[:, 2 * vp : 2 * vp + nv, :]
                    src = ps[:, 0 : nv * 256].rearrange("p (a b) -> p a b", a=nv)
                    if vp % 2 == 0:
                        nc.vector.tensor_copy(dst, src)
                    else:
                        nc.scalar.activation(
                            dst, src, mybir.ActivationFunctionType.Copy
                        )

                scp = [None] * NPAIR

                def score_mm(t):
                    pi = t // 2
                    if t % 2 == 0:
                        scp[pi] = sc_ps.tile(
                            [128, 2, H * C], FP32, tag="sc", name=f"sc_{pi}"
                        )
                    sc = scp[pi][:, t % 2]
                    for kc in range(2):
                        nc.tensor.matmul(
                            sc,
                            qtr[:, kc, t * 128 : (t + 1) * 128],
                            w2t[:, kc, :],
                            start=(kc == 0),
                            stop=(kc == 1),
                        )

                def softmax_pair(pi):
                    # batch-edge masks (first tile of pair0, last of pair3)
                    scpair = scp[pi]
                    if pi == 0 or pi == NPAIR - 1:
                        part = 0 if pi == 0 else 1
                        mb = AP(
                            sA[:].tensor,
                            sA[:].offset + part * AW + 508,
                            [[2 * AW, 128], [0, H], [1, C]],
                        )
                        nc.vector.tensor_add(
                            scpair[:, part].rearrange("p (h c) -> p h c", h=H),
                            scpair[:, part].rearrange("p (h c) -> p h c", h=H),
                            mb,
                        )
                    expp = work.tile([128, 2, H * C], BF16, tag="expp")
                    nc.scalar.activation(
                        expp[:], scpair[:], mybir.ActivationFunctionType.Exp
                    )
                    den = work.tile([128, 2 * H], FP32, tag="den")
                    nc.vector.tensor_reduce(
                        den[:],
                        expp[:].rearrange("p a (h c) -> p (a h) c", h=H),
                        axis=mybir.AxisListType.X,
                        op=mybir.AluOpType.add,
                    )
                    rden = work.tile([128, 2 * H], FP32, tag="rden")
                    nc.vector.reciprocal(rden[:], den[:])
                    pn = work.tile([128, 2, H * C], BF16, tag="pn")
                    rb = AP(
                        rden[:].tensor,
                        rden[:].offset,
                        [[2 * H, 128], [1, 2 * H], [0, C]],
                    )
                    # normalize on the otherwise-idle GpSimd engine
                    nc.gpsimd.tensor_mul(
                        pn[:].rearrange("p a (h c) -> p (a h) c", h=H),
                        expp[:].rearrange("p a (h c) -> p (a h) c", h=H),
                        rb,
                    )
                    # banded stage into host-zeroed DRAM pad (SP ring)
                    dst = AP(
                        pnpad_d.tensor,
                        pnpad_d.offset + pi * PW + 128,
                        [[PPW, 128], [SW, 2 * H], [1, C]],
                    )
                    nc.sync.dma_start(dst, pn[:])

                def transpose_pair(pi):
                    # skewed-src transpose: S^T[j, i] = pn[i, j - i]
                    src = AP(
                        pnpad_d.tensor,
                        pnpad_d.offset + pi * PW + 128,
                        [[PPW - 1, 128], [1, 2 * SBW]],
                    )
                    nc.sync.dma_start_transpose(
                        sta2[pi][:].rearrange("p a c i -> p (a c) i"), src
                    )

                outT_r = outT_d.rearrange("(c p) t -> p c t", p=128)
                xpair = [None]

                def tile_b(s):
                    # band matmuls (+ per-pair xt copy, per-mega out-proj)
                    pi, half = s // 2, s % 2
                    sta = sta2[pi][:, half]
                    if half == 0:
                        xpair[0] = x_ps.tile(
                            [128, 2, 256], FP32, tag="xv", name=f"xv_{pi}"
                        )
                    xps = xpair[0][:, half]
                    for h in range(H):
                        out_sl = xps[
                            64 * (h % 2) : 64 * (h % 2) + 64,
                            128 * (h // 2) : 128 * (h // 2) + 128,
                        ]
                        nc.tensor.matmul(
                            out_sl,
                            vpark[0:128, s, h * DK : (h + 1) * DK],
                            sta[0:128, 2 * h, :],
                            start=True,
                            stop=False,
                        )
                        nc.tensor.matmul(
                            out_sl,
                            vpark[0:62, s + 1, h * DK : (h + 1) * DK],
                            sta[0:62, 2 * h + 1, :],
                            start=False,
                            stop=True,
                        )
                    if half == 1:
                        # one DVE copy per pair: xps2 -> xt token chunks
                        xdst = AP(
                            xt[:].tensor,
                            xt[:].offset + 2 * pi * 128,
                            [[2 * TPC, 128], [128, 2], [TPC, 2], [1, 128]],
                        )
                        nc.vector.tensor_copy(xdst, xpair[0][:])
                        # out-proj + store for this 256-token mega
                        m = pi
                        for mc in range(2):
                            ps = ob_ps.tile([128, 256], FP32, tag="obig")
                            for kc in range(2):
                                nc.tensor.matmul(
                                    ps[:],
                                    wot[:, kc, mc * 128 : (mc + 1) * 128],
                                    xt[:, kc, m * 256 : (m + 1) * 256],
                                    start=(kc == 0),
                                    stop=(kc == 1),
                                )
                            if mc == 0:
                                nc.vector.tensor_copy(
                                    outsb[:, mc, m * 256 : (m + 1) * 256], ps[:]
                                )
                            else:
                                nc.scalar.activation(
                                    outsb[:, mc, m * 256 : (m + 1) * 256],
                                    ps[:],
                                    mybir.ActivationFunctionType.Copy,
                                )
                        nc.scalar.dma_start(
                            outT_r[:, :, m * 256 : (m + 1) * 256],
                            outsb[:, :, m * 256 : (m + 1) * 256],
                        )

                # ---- schedule ------------------------------------------
                stage1_mega(0)
                score_mm(0)
                stage1_mega(1)
                score_mm(1)
                softmax_pair(0)
                vpark_chunk(0)
                vpark_chunk(1)
                score_mm(2)
                score_mm(3)
                softmax_pair(1)
                transpose_pair(0)
                vpark_chunk(2)
                score_mm(4)
                score_mm(5)
                softmax_pair(2)
                transpose_pair(1)
                vpark_chunk(3)
                vpark_chunk(4)
                score_mm(6)
                score_mm(7)
                softmax_pair(3)
                transpose_pair(2)
                transpose_pair(3)
                for s in range(N_TILES):
                    tile_b(s)

    nc.compile()
    return nc


def _pack_weight_t(w, cols):
    """w [cols, NF] -> [128, 2, cols]: out[p, c, j] = w[j, c*128 + p]."""
    wt = np.ascontiguousarray(np.asarray(w, np.float32).T)  # [NF, cols]
    return np.ascontiguousarray(
        wt.reshape(2, 128, cols).transpose(1, 0, 2)
    )


def make_inputs(query, value, w1, w2, w3, w_out):
    """Host-side shard/transpose/cast. Returns per-core in_maps."""
    fq = np.asarray(query, np.float32).reshape(B * T, NF)
    fv = np.asarray(value, np.float32).reshape(B * T, NF)
    w1p = _pack_weight_t(w1, 256)
    w2p = _pack_weight_t(w2, 252)
    w3p = _pack_weight_t(w3, 256)
    wop = _pack_weight_t(w_out, 256)
    pnpad = np.zeros((128, PPW), NP_BF16)

    in_maps = []
    k = np.arange(C)
    for c in range(N_CORES):
        t0 = c * TPC
        b = (c * TPC) // T
        b0, b1 = b * T, (b + 1) * T
        qT = np.ascontiguousarray(fq[t0 : t0 + TPC].T)  # [256, 1024]
        qTp = np.ascontiguousarray(qT.reshape(2, 128, TPC).transpose(1, 0, 2))
        # parked value rows: global tokens [t0-31, t0-31+VPAD), zero outside
        vrows = np.zeros((VPAD, NF), np.float32)
        lo = t0 - HALF
        s0, s1 = max(lo, b0), min(lo + VPAD, b1)
        vrows[s0 - lo : s1 - lo] = fv[s0:s1]
        vT = np.ascontiguousarray(vrows.T)  # [256, VPAD]
        vTp = np.ascontiguousarray(vT.reshape(2, 128, VPAD).transpose(1, 0, 2))
        # additive band masks for first/last tile (batch edges only);
        # packed as [128, 2, 63]: [:, 0] = first-tile mask, [:, 1] = last-tile
        mask2 = np.zeros((128, 2, C), np.float32)
        g = t0 + np.arange(128)[:, None]
        bad = (g + k - HALF < b0) | (g + k - HALF >= b1)
        mask2[:, 0, :] = np.where(bad, -30000.0, 0.0)
        g = t0 + (N_TILES - 1) * 128 + np.arange(128)[:, None]
        bad = (g + k - HALF < b0) | (g + k - HALF >= b1)
        mask2[:, 1, :] = np.where(bad, -30000.0, 0.0)
        # mask2 packed at cols 508:571 of blobA, kc-slot a holds mask part a
        maskp = mask2.transpose(0, 1, 2)  # [128, 2, 63]

        blobA = np.concatenate(
            [w1p, w2p, maskp, qTp[:, :, 0:512]], axis=2
        ).astype(NP_BF16)
        blobC = np.ascontiguousarray(qTp[:, :, 512:TPC]).astype(NP_BF16)
        blobD = np.concatenate([w3p, vTp, wop], axis=2).astype(NP_BF16)
        in_maps.append(
            {
                "blobA": blobA,
                "blobC": blobC,
                "blobD": blobD,
                "pnpad": pnpad,
            }
        )
    return in_maps


_NC_CACHE = None


def kernel(query, key, value, mask, w1, w2, w3, w_out):
    global _NC_CACHE
    if _NC_CACHE is None:
        _NC_CACHE = build_program()
    nc = _NC_CACHE
    in_maps = make_inputs(query, value, w1, w2, w3, w_out)
    res = bass_utils.run_bass_kernel_spmd(nc, in_maps, core_ids=list(range(N_CORES)))
    outs = []
    for c in range(N_CORES):
        outT = res.results[c]["outT"]  # (256, 1024)
        outs.append(np.ascontiguousarray(outT.T))
    full = np.concatenate(outs, axis=0)  # (8192, 256)
    return full.reshape(B, T, NF).astype(np.float32)


# revision 8
# speedup vs baseline: 1.1129x; 1.1129x over previous
"""Trainium2 Bass kernel for LocalDenseSynthesizerAttention (band C=63, H=4 heads).

Sharding: 8192 tokens (B=2 x T=4096 flattened) split contiguously across 8
cores (1024 tokens each).  Each core runs an identical program on its own
slice; batch-edge band masking and value halo padding are handled host-side
via per-core input data, so the program is uniform SPMD.

Band construction (v3, scatter-free): the normalized softmax rows pn[i, h, k]
are DMA'd into a host-zeroed DRAM pad buffer at banded offsets
(pad[i, 128 + 1024*half + 256*h + k]), then a single DmaTransposeAnt per
tile-pair reads the pad with a SKEWED access pattern (partition step =
row_pitch - 1, i.e. element (i, f) = pad[i, 128 + f - i]) which realizes the
band skew S^T[j, i] = pn[i, j - i] and the transpose in one DMA.  Off-band
positions read host-provided zeros.  DRAM strides are unrestricted (the
partition-step legality check only applies to SBUF APs), and the tile
framework tracks the DRAM write->read hazard with semaphores.

Softmax runs at tile-PAIR granularity (256 tokens) to halve fixed per-op
engine overheads; the normalize multiply runs on the otherwise-idle GpSimd
engine.  Input DMAs ride the SP HWDGE ring in need-order (w1/w2/mask/qt0,
qt1, w3/v, v/wout); pn staging + transposes are interleaved behind them on
the same ring; per-mega output stores use the Act ring.
"""

import numpy as np
import ml_dtypes

import concourse.bass as bass
import concourse.bacc as bacc
import concourse.mybir as mybir
import concourse.tile as tile
from concourse.ap import AP
from concourse import bass_utils

BF16 = mybir.dt.bfloat16
FP32 = mybir.dt.float32
NP_BF16 = ml_dtypes.bfloat16

B, T, NF = 2, 4096, 256
H, C, DK = 4, 63, 64
HALF = (C - 1) // 2  # 31
N_CORES = 8
TPC = (B * T) // N_CORES  # 1024 tokens per core
N_TILES = TPC // 128  # 8
VPAD = 1152  # parked value rows: tokens [-31, 1121) relative to core start
SW = 256  # per-head section width in the band buffer
SBW = H * SW  # 1024
NCH = SBW // 128  # 8 chunks of S^T per tile
NPAIR = N_TILES // 2  # 4
PW = 128 + 2 * SBW  # 2176 pad width per tile-pair (128-col zero head)
PPW = NPAIR * PW  # 8704 total pnpad width
AW = 256 + 252 + 63 + 512  # blobA cols: w1t | w2t | mask | qt[0:512] = 1083


def build_program(reps: int = 1):
    import contextlib

    nc = bacc.Bacc(
        "TRN2",
        target_bir_lowering=False,
        debug=False,
        enable_asserts=False,
        num_devices=N_CORES,
    )

    blobA_d = nc.dram_tensor("blobA", [128, 2, AW], BF16, kind="ExternalInput").ap()
    blobC_d = nc.dram_tensor("blobC", [128, 2, 512], BF16, kind="ExternalInput").ap()
    blobD_d = nc.dram_tensor("blobD", [128, 2, 1664], BF16, kind="ExternalInput").ap()
    pnpad_d = [
        nc.dram_tensor(f"pnpad{i}", [128, PW], BF16, kind="ExternalInput").ap()
        for i in range(NPAIR)
    ]
    outT_d = [
        nc.dram_tensor(f"outT{m}", [NF, 256], BF16, kind="ExternalOutput").ap()
        for m in range(NPAIR)
    ]

    with tile.TileContext(nc) as tc:
        with (
            tc.tile_pool(name="inp", bufs=1) as inp,
            tc.tile_pool(name="work", bufs=6) as work,
            tc.tile_pool(name="big_ps", bufs=2, space="PSUM") as big_ps,
            tc.tile_pool(name="ob_ps", bufs=1, space="PSUM") as ob_ps,
            tc.tile_pool(name="sc_ps", bufs=3, space="PSUM") as sc_ps,
            tc.tile_pool(name="x_ps", bufs=2, space="PSUM") as x_ps,
        ):
            # ---- persistent SBUF tensors --------------------------------
            sA = inp.tile([128, 2, AW], BF16, tag="sA")
            sC = inp.tile([128, 2, 512], BF16, tag="sC")  # qt[512:1024]
            sD = inp.tile([128, 2, 1664], BF16, tag="sD")  # w3t | vt | wot
            w1t = sA[:, :, 0:256]
            w2t = sA[:, :, 256:508]
            w3t = sD[:, :, 0:256]
            vt = sD[:, :, 256:1408]
            wot = sD[:, :, 1408:1664]
            qtr = inp.tile([128, 2, TPC], BF16, tag="qtr")
            vpark = inp.tile([128, 9, NF], BF16, tag="vpark")
            xt = inp.tile([128, 2, TPC], BF16, tag="xt")
            outsb = inp.tile([128, 2, TPC], BF16, tag="outsb")
            sta2 = [
                inp.tile([128, 2, NCH, 128], BF16, tag=f"sta2_{i}", name=f"sta2_{i}")
                for i in range(NPAIR)
            ]

            loop_ctx = (
                tc.For_i(0, reps, 1, hint_engines=(mybir.EngineType.PE,))
                if reps > 1
                else contextlib.nullcontext()
            )
            with loop_ctx:
                # ---- input DMAs, all on the SP ring in need-order -------
                nc.sync.dma_start(sA[:], blobA_d)
                nc.sync.dma_start(sC[:], blobC_d)
                nc.sync.dma_start(sD[:, :, 0:832], blobD_d[:, :, 0:832])
                nc.sync.dma_start(sD[:, :, 832:1664], blobD_d[:, :, 832:1664])

                def stage1_mega(m):
                    # qtr[:, mc, m*512:(m+1)*512] = relu(w1 @ q) for 512 tokens
                    qsrc = sA[:, :, 571:1083] if m == 0 else sC
                    for mc in range(2):
                        ps = big_ps.tile([128, 512], FP32, tag="big")
                        for kc in range(2):
                            nc.tensor.matmul(
                                ps[:],
                                w1t[:, kc, mc * 128 : (mc + 1) * 128],
                                qsrc[:, kc, :],
                                start=(kc == 0),
                                stop=(kc == 1),
                            )
                        nc.scalar.activation(
                            qtr[:, mc, m * 512 : (m + 1) * 512],
                            ps[:],
                            mybir.ActivationFunctionType.Relu,
                        )

                def vpark_chunk(vp):
                    # V = value @ w3.T parked at -31 offset, sections 2vp, 2vp+1
                    nv = 2 if vp < 4 else 1
                    ps = big_ps.tile([128, 512], FP32, tag="big")
                    for j in range(nv):
                        vtile = 2 * vp + j
                        for kc in range(2):
                            nc.tensor.matmul(
                                ps[:, j * 256 : (j + 1) * 256],
                                vt[:, kc, vtile * 128 : (vtile + 1) * 128],
                                w3t[:, kc, :],
                                start=(kc == 0),
                                stop=(kc == 1),
                            )
                    dst = vpark[:, 2 * vp : 2 * vp + nv, :]
                    src = ps[:, 0 : nv * 256].rearrange("p (a b) -> p a b", a=nv)
                    if vp % 2 == 0:
                        nc.vector.tensor_copy(dst, src)
                    else:
                        nc.scalar.activation(
                            dst, src, mybir.ActivationFunctionType.Copy
                        )

                scp = [None] * NPAIR

                def score_mm(t):
                    pi = t // 2
                    if t % 2 == 0:
                        scp[pi] = sc_ps.tile(
                            [128, 2, H * C], FP32, tag="sc", name=f"sc_{pi}"
                        )
                    sc = scp[pi][:, t % 2]
                    for kc in range(2):
                        nc.tensor.matmul(
                            sc,
                            qtr[:, kc, t * 128 : (t + 1) * 128],
                            w2t[:, kc, :],
                            start=(kc == 0),
                            stop=(kc == 1),
                        )

                def softmax_pair(pi):
                    # batch-edge masks (first tile of pair0, last of pair3)
                    scpair = scp[pi]
                    if pi == 0 or pi == NPAIR - 1:
                        part = 0 if pi == 0 else 1
                        mb = AP(
                            sA[:].tensor,
                            sA[:].offset + part * AW + 508,
                            [[2 * AW, 128], [0, H], [1, C]],
                        )
                        nc.vector.tensor_add(
                            scpair[:, part].rearrange("p (h c) -> p h c", h=H),
                            scpair[:, part].rearrange("p (h c) -> p h c", h=H),
                            mb,
                        )
                    expp = work.tile([128, 2, H * C], BF16, tag="expp")
                    nc.scalar.activation(
                        expp[:], scpair[:], mybir.ActivationFunctionType.Exp
                    )
                    den = work.tile([128, 2 * H], FP32, tag="den")
                    nc.vector.tensor_reduce(
                        den[:],
                        expp[:].rearrange("p a (h c) -> p (a h) c", h=H),
                        axis=mybir.AxisListType.X,
                        op=mybir.AluOpType.add,
                    )
                    rden = work.tile([128, 2 * H], FP32, tag="rden")
                    nc.vector.reciprocal(rden[:], den[:])
                    pn = work.tile([128, 2, H * C], BF16, tag="pn")
                    rb = AP(
                        rden[:].tensor,
                        rden[:].offset,
                        [[2 * H, 128], [1, 2 * H], [0, C]],
                    )
                    # normalize on the otherwise-idle GpSimd engine
                    nc.gpsimd.tensor_mul(
                        pn[:].rearrange("p a (h c) -> p (a h) c", h=H),
                        expp[:].rearrange("p a (h c) -> p (a h) c", h=H),
                        rb,
                    )
                    # banded stage into host-zeroed DRAM pad (SP ring)
                    dst = AP(
                        pnpad_d[pi].tensor,
                        pnpad_d[pi].offset + 128,
                        [[PW, 128], [SW, 2 * H], [1, C]],
                    )
                    nc.sync.dma_start(dst, pn[:])

                def transpose_pair(pi):
                    # skewed-src transpose: S^T[j, i] = pn[i, j - i]
                    src = AP(
                        pnpad_d[pi].tensor,
                        pnpad_d[pi].offset + 128,
                        [[PW - 1, 128], [1, 2 * SBW]],
                    )
                    nc.sync.dma_start_transpose(
                        sta2[pi][:].rearrange("p a c i -> p (a c) i"), src
                    )

                outT_r = [
                    d.rearrange("(c p) t -> p c t", p=128) for d in outT_d
                ]
                xpair = [None]

                def tile_b(s):
                    # band matmuls (+ per-pair xt copy, per-mega out-proj)
                    pi, half = s // 2, s % 2
                    sta = sta2[pi][:, half]
                    if half == 0:
                        xpair[0] = x_ps.tile(
                            [128, 2, 256], FP32, tag="xv", name=f"xv_{pi}"
                        )
                    xps = xpair[0][:, half]
                    for h in range(H):
                        out_sl = xps[
                            64 * (h % 2) : 64 * (h % 2) + 64,
                            128 * (h // 2) : 128 * (h // 2) + 128,
                        ]
                        nc.tensor.matmul(
                            out_sl,
                            vpark[0:128, s, h * DK : (h + 1) * DK],
                            sta[0:128, 2 * h, :],
                            start=True,
                            stop=False,
                        )
                        nc.tensor.matmul(
                            out_sl,
                            vpark[0:62, s + 1, h * DK : (h + 1) * DK],
                            sta[0:62, 2 * h + 1, :],
                            start=False,
                            stop=True,
                        )
                    if half == 1:
                        # one DVE copy per pair: xps2 -> xt token chunks
                        xdst = AP(
                            xt[:].tensor,
                            xt[:].offset + 2 * pi * 128,
                            [[2 * TPC, 128], [128, 2], [TPC, 2], [1, 128]],
                        )
                        nc.vector.tensor_copy(xdst, xpair[0][:])
                        # out-proj + store for this 256-token mega
                        m = pi
                        for mc in range(2):
                            ps = ob_ps.tile([128, 256], FP32, tag="obig")
                            for kc in range(2):
                                nc.tensor.matmul(
                                    ps[:],
                                    wot[:, kc, mc * 128 : (mc + 1) * 128],
                                    xt[:, kc, m * 256 : (m + 1) * 256],
                                    start=(kc == 0),
                                    stop=(kc == 1),
                                )
                            if mc == 0:
                                nc.vector.tensor_copy(
                                    outsb[:, mc, m * 256 : (m + 1) * 256], ps[:]
                                )
                            else:
                                nc.scalar.activation(
                                    outsb[:, mc, m * 256 : (m + 1) * 256],
                                    ps[:],
                                    mybir.ActivationFunctionType.Copy,
                                )
                        nc.scalar.dma_start(
                            outT_r[m],
                            outsb[:, :, m * 256 : (m + 1) * 256],
                        )

                # ---- schedule ------------------------------------------
                stage1_mega(0)
                stage1_mega(1)
                score_mm(0)
                score_mm(1)
                softmax_pair(0)
                vpark_chunk(0)
                vpark_chunk(1)
                score_mm(2)
                score_mm(3)
                softmax_pair(1)
                transpose_pair(0)
                vpark_chunk(2)
                score_mm(4)
                score_mm(5)
                softmax_pair(2)
                transpose_pair(1)
                vpark_chunk(3)
                vpark_chunk(4)
                score_mm(6)
                score_mm(7)
                softmax_pair(3)
                transpose_pair(2)
                transpose_pair(3)
                for s in range(N_TILES):
                    tile_b(s)

    nc.compile()
    return nc


def _pack_weight_t(w, cols):
    """w [cols, NF] -> [128, 2, cols]: out[p, c, j] = w[j, c*128 + p]."""
    wt = np.ascontiguousarray(np.asarray(w, np.float32).T)  # [NF, cols]
    return np.ascontiguousarray(
        wt.reshape(2, 128, cols).transpose(1, 0, 2)
    )


def make_inputs(query, value, w1, w2, w3, w_out):
    """Host-side shard/transpose/cast. Returns per-core in_maps."""
    fq = np.asarray(query, np.float32).reshape(B * T, NF)
    fv = np.asarray(value, np.float32).reshape(B * T, NF)
    w1p = _pack_weight_t(w1, 256)
    w2p = _pack_weight_t(w2, 252)
    w3p = _pack_weight_t(w3, 256)
    wop = _pack_weight_t(w_out, 256)
    pnpad = np.zeros((128, PW), NP_BF16)

    in_maps = []
    k = np.arange(C)
    for c in range(N_CORES):
        t0 = c * TPC
        b = (c * TPC) // T
        b0, b1 = b * T, (b + 1) * T
        qT = np.ascontiguousarray(fq[t0 : t0 + TPC].T)  # [256, 1024]
        qTp = np.ascontiguousarray(qT.reshape(2, 128, TPC).transpose(1, 0, 2))
        # parked value rows: global tokens [t0-31, t0-31+VPAD), zero outside
        vrows = np.zeros((VPAD, NF), np.float32)
        lo = t0 - HALF
        s0, s1 = max(lo, b0), min(lo + VPAD, b1)
        vrows[s0 - lo : s1 - lo] = fv[s0:s1]
        vT = np.ascontiguousarray(vrows.T)  # [256, VPAD]
        vTp = np.ascontiguousarray(vT.reshape(2, 128, VPAD).transpose(1, 0, 2))
        # additive band masks for first/last tile (batch edges only);
        # packed as [128, 2, 63]: [:, 0] = first-tile mask, [:, 1] = last-tile
        mask2 = np.zeros((128, 2, C), np.float32)
        g = t0 + np.arange(128)[:, None]
        bad = (g + k - HALF < b0) | (g + k - HALF >= b1)
        mask2[:, 0, :] = np.where(bad, -30000.0, 0.0)
        g = t0 + (N_TILES - 1) * 128 + np.arange(128)[:, None]
        bad = (g + k - HALF < b0) | (g + k - HALF >= b1)
        mask2[:, 1, :] = np.where(bad, -30000.0, 0.0)
        # mask2 packed at cols 508:571 of blobA, kc-slot a holds mask part a
        maskp = mask2.transpose(0, 1, 2)  # [128, 2, 63]

        blobA = np.concatenate(
            [w1p, w2p, maskp, qTp[:, :, 0:512]], axis=2
        ).astype(NP_BF16)
        blobC = np.ascontiguousarray(qTp[:, :, 512:TPC]).astype(NP_BF16)
        blobD = np.concatenate([w3p, vTp, wop], axis=2).astype(NP_BF16)
        imap = {"blobA": blobA, "blobC": blobC, "blobD": blobD}
        for i in range(NPAIR):
            imap[f"pnpad{i}"] = pnpad
        in_maps.append(imap)
    return in_maps


_NC_CACHE = None


def kernel(query, key, value, mask, w1, w2, w3, w_out):
    global _NC_CACHE
    if _NC_CACHE is None:
        _NC_CACHE = build_program()
    nc = _NC_CACHE
    in_maps = make_inputs(query, value, w1, w2, w3, w_out)
    res = bass_utils.run_bass_kernel_spmd(nc, in_maps, core_ids=list(range(N_CORES)))
    outs = []
    for c in range(N_CORES):
        outT = np.concatenate(
            [res.results[c][f"outT{m}"] for m in range(NPAIR)], axis=1
        )  # (256, 1024)
        outs.append(np.ascontiguousarray(outT.T))
    full = np.concatenate(outs, axis=0)  # (8192, 256)
    return full.reshape(B, T, NF).astype(np.float32)


# revision 9
# speedup vs baseline: 1.1989x; 1.0773x over previous
"""Trainium2 Bass kernel for LocalDenseSynthesizerAttention (band C=63, H=4 heads).

Sharding: 8192 tokens (B=2 x T=4096 flattened) split contiguously across 8
cores (1024 tokens each).  Each core runs an identical program on its own
slice; batch-edge band masking and value halo padding are handled host-side
via per-core input data, so the program is uniform SPMD.

Band construction (v3, scatter-free): the normalized softmax rows pn[i, h, k]
are DMA'd into a host-zeroed DRAM pad buffer at banded offsets
(pad[i, 128 + 1024*half + 256*h + k]), then a single DmaTransposeAnt per
tile-pair reads the pad with a SKEWED access pattern (partition step =
row_pitch - 1, i.e. element (i, f) = pad[i, 128 + f - i]) which realizes the
band skew S^T[j, i] = pn[i, j - i] and the transpose in one DMA.  Off-band
positions read host-provided zeros.  DRAM strides are unrestricted (the
partition-step legality check only applies to SBUF APs), and the tile
framework tracks the DRAM write->read hazard with semaphores.

Softmax runs at tile-PAIR granularity (256 tokens) to halve fixed per-op
engine overheads; the normalize multiply runs on the otherwise-idle GpSimd
engine.  Input DMAs ride the SP HWDGE ring in need-order (w1/w2/mask/qt0,
qt1, w3/v, v/wout); pn staging + transposes are interleaved behind them on
the same ring; per-mega output stores use the Act ring.
"""

import numpy as np
import ml_dtypes

import concourse.bass as bass
import concourse.bacc as bacc
import concourse.mybir as mybir
import concourse.tile as tile
from concourse.ap import AP
from concourse import bass_utils

BF16 = mybir.dt.bfloat16
FP32 = mybir.dt.float32
NP_BF16 = ml_dtypes.bfloat16

B, T, NF = 2, 4096, 256
H, C, DK = 4, 63, 64
HALF = (C - 1) // 2  # 31
N_CORES = 8
TPC = (B * T) // N_CORES  # 1024 tokens per core
N_TILES = TPC // 128  # 8
VPAD = 1152  # parked value rows: tokens [-31, 1121) relative to core start
SW = 256  # per-head section width in the band buffer
SBW = H * SW  # 1024
NCH = SBW // 128  # 8 chunks of S^T per tile
NPAIR = N_TILES // 2  # 4
PW = 128 + 2 * SBW  # 2176 pad width per tile-pair (128-col zero head)
PPW = NPAIR * PW  # 8704 total pnpad width
AW = 256 + 252 + 63 + 512  # blobA cols: w1t | w2t | mask | qt[0:512] = 1083


def build_program(reps: int = 1):
    import contextlib

    nc = bacc.Bacc(
        "TRN2",
        target_bir_lowering=False,
        debug=False,
        enable_asserts=False,
        num_devices=N_CORES,
    )

    blobA_d = nc.dram_tensor("blobA", [128, 2, AW], BF16, kind="ExternalInput").ap()
    blobC_d = nc.dram_tensor("blobC", [128, 2, 512], BF16, kind="ExternalInput").ap()
    blobD_d = nc.dram_tensor("blobD", [128, 2, 1664], BF16, kind="ExternalInput").ap()
    pnpad_d = [
        nc.dram_tensor(f"pnpad{i}", [128, PW], BF16, kind="ExternalInput").ap()
        for i in range(NPAIR)
    ]
    outT_d = [
        nc.dram_tensor(f"outT{m}", [NF, 512], BF16, kind="ExternalOutput").ap()
        for m in range(2)
    ]

    with tile.TileContext(nc) as tc:
        with (
            tc.tile_pool(name="inp", bufs=1) as inp,
            tc.tile_pool(name="work", bufs=6) as work,
            tc.tile_pool(name="big_ps", bufs=2, space="PSUM") as big_ps,
            tc.tile_pool(name="ob_ps", bufs=1, space="PSUM") as ob_ps,
            tc.tile_pool(name="sc_ps", bufs=3, space="PSUM") as sc_ps,
            tc.tile_pool(name="x_ps", bufs=2, space="PSUM") as x_ps,
        ):
            # ---- persistent SBUF tensors --------------------------------
            sA = inp.tile([128, 2, AW], BF16, tag="sA")
            sC = inp.tile([128, 2, 512], BF16, tag="sC")  # qt[512:1024]
            sD = inp.tile([128, 2, 1664], BF16, tag="sD")  # w3t | vt | wot
            w1t = sA[:, :, 0:256]
            w2t = sA[:, :, 256:508]
            w3t = sD[:, :, 0:256]
            vt = sD[:, :, 256:1408]
            wot = sD[:, :, 1408:1664]
            qtr = inp.tile([128, 2, TPC], BF16, tag="qtr")
            vpark = inp.tile([128, 9, NF], BF16, tag="vpark")
            xt = inp.tile([128, 2, TPC], BF16, tag="xt")
            outsb = inp.tile([128, 2, TPC], BF16, tag="outsb")
            sta2 = [
                inp.tile([128, 2, NCH, 128], BF16, tag=f"sta2_{i}", name=f"sta2_{i}")
                for i in range(NPAIR)
            ]

            loop_ctx = (
                tc.For_i(0, reps, 1, hint_engines=(mybir.EngineType.PE,))
                if reps > 1
                else contextlib.nullcontext()
            )
            with loop_ctx:
                # ---- input DMAs, all on the SP ring in need-order -------
                nc.sync.dma_start(sA[:], blobA_d)
                nc.sync.dma_start(sC[:], blobC_d)
                nc.sync.dma_start(sD[:], blobD_d)

                def stage1_mega(m):
                    # qtr[:, mc, m*512:(m+1)*512] = relu(w1 @ q) for 512 tokens
                    qsrc = sA[:, :, 571:1083] if m == 0 else sC
                    for mc in range(2):
                        ps = big_ps.tile([128, 512], FP32, tag="big")
                        for kc in range(2):
                            nc.tensor.matmul(
                                ps[:],
                                w1t[:, kc, mc * 128 : (mc + 1) * 128],
                                qsrc[:, kc, :],
                                start=(kc == 0),
                                stop=(kc == 1),
                            )
                        nc.scalar.activation(
                            qtr[:, mc, m * 512 : (m + 1) * 512],
                            ps[:],
                            mybir.ActivationFunctionType.Relu,
                        )

                def vpark_chunk(vp):
                    # V = value @ w3.T parked at -31 offset, sections 2vp, 2vp+1
                    nv = 2 if vp < 4 else 1
                    ps = big_ps.tile([128, 512], FP32, tag="big")
                    for j in range(nv):
                        vtile = 2 * vp + j
                        for kc in range(2):
                            nc.tensor.matmul(
                                ps[:, j * 256 : (j + 1) * 256],
                                vt[:, kc, vtile * 128 : (vtile + 1) * 128],
                                w3t[:, kc, :],
                                start=(kc == 0),
                                stop=(kc == 1),
                            )
                    dst = vpark[:, 2 * vp : 2 * vp + nv, :]
                    src = ps[:, 0 : nv * 256].rearrange("p (a b) -> p a b", a=nv)
                    if vp % 2 == 0:
                        nc.vector.tensor_copy(dst, src)
                    else:
                        nc.scalar.activation(
                            dst, src, mybir.ActivationFunctionType.Copy
                        )

                scp = [None] * NPAIR

                def score_mm(t):
                    pi = t // 2
                    if t % 2 == 0:
                        scp[pi] = sc_ps.tile(
                            [128, 2, H * C], FP32, tag="sc", name=f"sc_{pi}"
                        )
                    sc = scp[pi][:, t % 2]
                    for kc in range(2):
                        nc.tensor.matmul(
                            sc,
                            qtr[:, kc, t * 128 : (t + 1) * 128],
                            w2t[:, kc, :],
                            start=(kc == 0),
                            stop=(kc == 1),
                        )

                pnbuf = [None] * NPAIR

                def softmax_tile(t):
                    # per-tile softmax; stage the pair's pn once both halves
                    # are normalized into the shared pn pair buffer
                    pi, half = t // 2, t % 2
                    sc = scp[pi][:, half]
                    if t == 0 or t == N_TILES - 1:
                        part = 0 if t == 0 else 1
                        mb = AP(
                            sA[:].tensor,
                            sA[:].offset + part * AW + 508,
                            [[2 * AW, 128], [0, H], [1, C]],
                        )
                        nc.vector.tensor_add(
                            sc.rearrange("p (h c) -> p h c", h=H),
                            sc.rearrange("p (h c) -> p h c", h=H),
                            mb,
                        )
                    expp = work.tile(
                        [128, H * C], BF16, tag="expp", name=f"expp_{t}"
                    )
                    nc.scalar.activation(
                        expp[:], sc, mybir.ActivationFunctionType.Exp
                    )
                    den = work.tile([128, H], FP32, tag="den", name=f"den_{t}")
                    nc.vector.tensor_reduce(
                        den[:],
                        expp[:].rearrange("p (h c) -> p h c", h=H),
                        axis=mybir.AxisListType.X,
                        op=mybir.AluOpType.add,
                    )
                    rden = work.tile([128, H], FP32, tag="rden", name=f"rden_{t}")
                    nc.vector.reciprocal(rden[:], den[:])
                    if half == 0:
                        pnbuf[pi] = work.tile(
                            [128, 2, H * C], BF16, tag="pn", name=f"pn_{pi}"
                        )
                    pn = pnbuf[pi]
                    rb = AP(
                        rden[:].tensor, rden[:].offset, [[H, 128], [1, H], [0, C]]
                    )
                    # normalize on the otherwise-idle GpSimd engine
                    nc.gpsimd.tensor_mul(
                        pn[:, half].rearrange("p (h c) -> p h c", h=H),
                        expp[:].rearrange("p (h c) -> p h c", h=H),
                        rb,
                    )
                    if half == 1:
                        # banded stage into host-zeroed DRAM pad (SP ring)
                        dst = AP(
                            pnpad_d[pi].tensor,
                            pnpad_d[pi].offset + 128,
                            [[PW, 128], [SW, 2 * H], [1, C]],
                        )
                        nc.sync.dma_start(dst, pn[:])

                def transpose_pair(pi):
                    # skewed-src transpose: S^T[j, i] = pn[i, j - i]
                    src = AP(
                        pnpad_d[pi].tensor,
                        pnpad_d[pi].offset + 128,
                        [[PW - 1, 128], [1, 2 * SBW]],
                    )
                    nc.sync.dma_start_transpose(
                        sta2[pi][:].rearrange("p a c i -> p (a c) i"), src
                    )

                outT_r = [
                    d.rearrange("(c p) t -> p c t", p=128) for d in outT_d
                ]
                xpair = [None]

                def tile_b(s):
                    # band matmuls (+ per-pair xt copy, per-mega out-proj)
                    pi, half = s // 2, s % 2
                    sta = sta2[pi][:, half]
                    if half == 0:
                        xpair[0] = x_ps.tile(
                            [128, 2, 256], FP32, tag="xv", name=f"xv_{pi}"
                        )
                    xps = xpair[0][:, half]
                    for h in range(H):
                        out_sl = xps[
                            64 * (h % 2) : 64 * (h % 2) + 64,
                            128 * (h // 2) : 128 * (h // 2) + 128,
                        ]
                        nc.tensor.matmul(
                            out_sl,
                            vpark[0:128, s, h * DK : (h + 1) * DK],
                            sta[0:128, 2 * h, :],
                            start=True,
                            stop=False,
                        )
                        nc.tensor.matmul(
                            out_sl,
                            vpark[0:62, s + 1, h * DK : (h + 1) * DK],
                            sta[0:62, 2 * h + 1, :],
                            start=False,
                            stop=True,
                        )
                    if half == 1:
                        # one DVE copy per pair: xps2 -> xt token chunks
                        xdst = AP(
                            xt[:].tensor,
                            xt[:].offset + 2 * pi * 128,
                            [[2 * TPC, 128], [128, 2], [TPC, 2], [1, 128]],
                        )
                        nc.vector.tensor_copy(xdst, xpair[0][:])
                        # out-proj + store for this 256-token mega
                        m = pi
                        for mc in range(2):
                            ps = ob_ps.tile([128, 256], FP32, tag="obig")
                            for kc in range(2):
                                nc.tensor.matmul(
                                    ps[:],
                                    wot[:, kc, mc * 128 : (mc + 1) * 128],
                                    xt[:, kc, m * 256 : (m + 1) * 256],
                                    start=(kc == 0),
                                    stop=(kc == 1),
                                )
                            if mc == 0:
                                nc.vector.tensor_copy(
                                    outsb[:, mc, m * 256 : (m + 1) * 256], ps[:]
                                )
                            else:
                                nc.scalar.activation(
                                    outsb[:, mc, m * 256 : (m + 1) * 256],
                                    ps[:],
                                    mybir.ActivationFunctionType.Copy,
                                )
                        if m % 2 == 1:
                            nc.scalar.dma_start(
                                outT_r[m // 2],
                                outsb[:, :, (m - 1) * 256 : (m + 1) * 256],
                            )

                # ---- schedule ------------------------------------------
                stage1_mega(0)
                stage1_mega(1)
                score_mm(0)
                score_mm(1)
                softmax_tile(0)
                softmax_tile(1)
                score_mm(2)
                score_mm(3)
                softmax_tile(2)
                softmax_tile(3)
                vpark_chunk(0)
                vpark_chunk(1)
                score_mm(4)
                score_mm(5)
                softmax_tile(4)
                softmax_tile(5)
                transpose_pair(0)
                vpark_chunk(2)
                score_mm(6)
                score_mm(7)
                softmax_tile(6)
                softmax_tile(7)
                transpose_pair(1)
                vpark_chunk(3)
                vpark_chunk(4)
                transpose_pair(2)
                transpose_pair(3)
                for s in range(N_TILES):
                    tile_b(s)

    nc.compile()
    return nc


def _pack_weight_t(w, cols):
    """w [cols, NF] -> [128, 2, cols]: out[p, c, j] = w[j, c*128 + p]."""
    wt = np.ascontiguousarray(np.asarray(w, np.float32).T)  # [NF, cols]
    return np.ascontiguousarray(
        wt.reshape(2, 128, cols).transpose(1, 0, 2)
    )


def make_inputs(query, value, w1, w2, w3, w_out):
    """Host-side shard/transpose/cast. Returns per-core in_maps."""
    fq = np.asarray(query, np.float32).reshape(B * T, NF)
    fv = np.asarray(value, np.float32).reshape(B * T, NF)
    w1p = _pack_weight_t(w1, 256)
    w2p = _pack_weight_t(w2, 252)
    w3p = _pack_weight_t(w3, 256)
    wop = _pack_weight_t(w_out, 256)
    pnpad = np.zeros((128, PW), NP_BF16)

    in_maps = []
    k = np.arange(C)
    for c in range(N_CORES):
        t0 = c * TPC
        b = (c * TPC) // T
        b0, b1 = b * T, (b + 1) * T
        qT = np.ascontiguousarray(fq[t0 : t0 + TPC].T)  # [256, 1024]
        qTp = np.ascontiguousarray(qT.reshape(2, 128, TPC).transpose(1, 0, 2))
        # parked value rows: global tokens [t0-31, t0-31+VPAD), zero outside
        vrows = np.zeros((VPAD, NF), np.float32)
        lo = t0 - HALF
        s0, s1 = max(lo, b0), min(lo + VPAD, b1)
        vrows[s0 - lo : s1 - lo] = fv[s0:s1]
        vT = np.ascontiguousarray(vrows.T)  # [256, VPAD]
        vTp = np.ascontiguousarray(vT.reshape(2, 128, VPAD).transpose(1, 0, 2))
        # additive band masks for first/last tile (batch edges only);
        # packed as [128, 2, 63]: [:, 0] = first-tile mask, [:, 1] = last-tile
        mask2 = np.zeros((128, 2, C), np.float32)
        g = t0 + np.arange(128)[:, None]
        bad = (g + k - HALF < b0) | (g + k - HALF >= b1)
        mask2[:, 0, :] = np.where(bad, -30000.0, 0.0)
        g = t0 + (N_TILES - 1) * 128 + np.arange(128)[:, None]
        bad = (g + k - HALF < b0) | (g + k - HALF >= b1)
        mask2[:, 1, :] = np.where(bad, -30000.0, 0.0)
        # mask2 packed at cols 508:571 of blobA, kc-slot a holds mask part a
        maskp = mask2.transpose(0, 1, 2)  # [128, 2, 63]

        blobA = np.concatenate(
            [w1p, w2p, maskp, qTp[:, :, 0:512]], axis=2
        ).astype(NP_BF16)
        blobC = np.ascontiguousarray(qTp[:, :, 512:TPC]).astype(NP_BF16)
        blobD = np.concatenate([w3p, vTp, wop], axis=2).astype(NP_BF16)
        imap = {"blobA": blobA, "blobC": blobC, "blobD": blobD}
        for i in range(NPAIR):
            imap[f"pnpad{i}"] = pnpad
        in_maps.append(imap)
    return in_maps


_NC_CACHE = None


def kernel(query, key, value, mask, w1, w2, w3, w_out):
    global _NC_CACHE
    if _NC_CACHE is None:
        _NC_CACHE = build_program()
    nc = _NC_CACHE
    in_maps = make_inputs(query, value, w1, w2, w3, w_out)
    res = bass_utils.run_bass_kernel_spmd(nc, in_maps, core_ids=list(range(N_CORES)))
    outs = []
    for c in range(N_CORES):
        outT = np.concatenate(
            [res.results[c][f"outT{m}"] for m in range(2)], axis=1
        )  # (256, 1024)
        outs.append(np.ascontiguousarray(outT.T))
    full = np.concatenate(outs, axis=0)  # (8192, 256)
    return full.reshape(B, T, NF).astype(np.float32)
